# revision 61
# baseline (speedup 1.0000x reference)
"""Head-parallel multi-head attention kernel for 8 TRN2 NeuronCores.

Problem: X[4096,1024] @ per-head Wq/Wk/Wv[1024,128] (+bias) -> per-head
scores S = q k^T * SCALE, softmax over the QUERY axis (axis n), z = attn @ v,
concat heads, sigmoid.  H=8 heads -> 1 head per core, zero collectives.

Per-core algorithm (head h):
  - Work in transposed space: T = S^T laid out [m(part), n(free)] so the
    softmax reduction is a free-axis row-sum.
  - softmax normalization folded into v:  z^T[e,n] = sum_m exp(T[m,n]) *
    (v[m,e]/denom[m]); denom comes free from the ACT accum_out of the exp.
  - X is transposed on-chip with regular matmuls against identity (fast
    path + engages the HAM clock), XT and expT stored fp8 for SBUF room.
  - The first SWEEP_MC m-chunks are processed column-chunk-outer so their
    exps start while X is still streaming in; the rest run m-chunk-outer
    in blocks whose AV matmuls hide under the next block's exps.
  - zT accumulated in PSUM per block, flushed to SBUF by DVE; sigmoid +
    transpose-back + store fused into the last block's AV sweep.
Host: shard weights per head, replicate X, concat core outputs on axis=1.
"""

import numpy as np
import ml_dtypes

from concourse import bacc, bass, tile, mybir
from concourse.bass_utils import run_bass_kernel_spmd

N, D, E = 4096, 1024, 128
H = 8
SCALE = 0.08838834764831845
BF16 = mybir.dt.bfloat16
F32 = mybir.dt.float32
FP8 = mybir.dt.float8e4

DC = D // 128      # 8 d-chunks of 128 (contraction tiles)
NG = N // 512      # 8 column groups of 512
MC = N // 128      # 32 m-chunks of 128
CH = N // 1024     # 4 exp chunks of 1024 per m-chunk
NQ = N // 512      # 8 AV n-chunks of 512
SWEEP_MC = 14      # m-chunks processed column-outer during the X stream

Exp = mybir.ActivationFunctionType.Exp
Sigmoid = mybir.ActivationFunctionType.Sigmoid
ADD = mybir.AluOpType.add
AX = mybir.AxisListType.X
PSUM = bass.MemorySpace.PSUM


def build():
    nc = bacc.Bacc("TRN2", target_bir_lowering=False, debug=False, num_devices=H)

    x_d = nc.dram_tensor("x", [N, D], F32, kind="ExternalInput")
    wq_d = nc.dram_tensor("wq", [D, E], F32, kind="ExternalInput")
    wk_d = nc.dram_tensor("wk", [D, E], F32, kind="ExternalInput")
    wv_d = nc.dram_tensor("wv", [D, E], F32, kind="ExternalInput")
    bq_d = nc.dram_tensor("bq", [E, 1], F32, kind="ExternalInput")
    bk_d = nc.dram_tensor("bk", [E, 1], F32, kind="ExternalInput")
    bv_d = nc.dram_tensor("bv", [E, 1], F32, kind="ExternalInput")
    out_d = nc.dram_tensor("out", [N, E], F32, kind="ExternalOutput")

    eye_bf_d = nc.inline_tensor(np.eye(128, dtype=ml_dtypes.bfloat16), "eye_bf")

    with tile.TileContext(nc) as tc:
        with (
            tc.tile_pool(name="persist", bufs=1) as persist,
            tc.tile_pool(name="expt", bufs=1) as expp,
            tc.tile_pool(name="vsp", bufs=32) as vsp,
            tc.tile_pool(name="dsp", bufs=1) as dsp,
        ):
            eye_bf = persist.tile([128, 128], BF16, tag="eye_bf")
            nc.gpsimd.dma_start(eye_bf[:], eye_bf_d[:])
            b_sbs = []
            for name, b_d in (("bq", bq_d), ("bk", bk_d), ("bv", bv_d)):
                b_sb = persist.tile([E, 1], F32, tag=name)
                nc.gpsimd.dma_start(b_sb[:], b_d[:])
                b_sbs.append(b_sb)

            qT = persist.tile([E, N], BF16, tag="qT")
            kT = persist.tile([E, N], BF16, tag="kT")
            vT = persist.tile([E, N], BF16, tag="vT")
            v_sb = persist.tile([128, MC, E], BF16, tag="v")
            zT = persist.tile([E, N], F32, tag="zT")
            projT = (qT, kT, vT)

            expts = {}
            recs = {}
            vss = {}

            def emit_t_exp(Tp, mc, chs, etag, ebufs, pool=None):
                """score^T matmuls + exp (fused row-sum) for chunks chs of mc."""
                if mc not in expts:
                    expts[mc] = (pool or expp).tile(
                        [128, N], FP8, tag=etag, name=f"et{mc}", bufs=ebufs
                    )
                    dst = dsp.tile([128, CH], F32, tag="ds", name=f"ds{mc}", bufs=40)
                    recs[mc] = (dst, None)
                et = expts[mc]
                dst = recs[mc][0]
                for ch in chs:
                    tp = Tp.tile([128, 1024], F32, tag="T", name=f"T{mc}_{ch}")
                    for h2 in range(2):
                        nc.tensor.matmul(
                            tp[:, h2 * 512 : (h2 + 1) * 512],
                            kT[:, mc * 128 : (mc + 1) * 128],
                            qT[:, ch * 1024 + h2 * 512 : ch * 1024 + (h2 + 1) * 512],
                            start=True,
                            stop=True,
                        )
                    nc.scalar.activation(
                        et[:, ch * 1024 : (ch + 1) * 1024],
                        tp[:],
                        Exp,
                        scale=SCALE,
                        accum_out=dst[:, ch : ch + 1],
                    )

            def emit_denom(mc):
                dst = recs[mc][0]
                den = dsp.tile([128, 1], F32, tag="den", name=f"den{mc}", bufs=8)
                nc.vector.tensor_reduce(den[:], dst[:], AX, ADD)
                rec = dsp.tile([128, 1], F32, tag="rec", name=f"rec{mc}", bufs=40)
                nc.vector.reciprocal(rec[:], den[:])
                recs[mc] = (dst, rec)

            def emit_vscale(mcs):
                for mc in mcs:
                    vs = vsp.tile([128, E], BF16, tag="vs", name=f"vs{mc}")
                    nc.vector.tensor_scalar_mul(vs[:], v_sb[:, mc, :], recs[mc][1][:])
                    vss[mc] = vs

            # ---------------- phase 1 + sweep era ----------------
            x_view = x_d.ap().rearrange("(g t p) d -> g p t d", p=128, t=4)
            with (
                tc.tile_pool(name="xload", bufs=1) as xload,
                tc.tile_pool(name="xtp", bufs=1) as xtp,
                tc.tile_pool(name="sweepTp", bufs=2, space=PSUM) as sweepTp,
                tc.tile_pool(name="trps", bufs=2, space=PSUM) as trp,
                tc.tile_pool(name="pjps", bufs=2, space=PSUM) as pjp,
            ):
                w_sbs = [None, None, None]
                w_ring = {0: nc.gpsimd, 1: nc.scalar, 2: nc.scalar}
                w_names = ("wq", "wk", "wv")
                w_drams = (wq_d, wk_d, wv_d)
                with tc.tile_pool(name="wload", bufs=1) as wload:
                    for i in range(3):
                        w_f = wload.tile(
                            [128, DC, E], F32, tag=f"wf{i}", name=f"wf{i}"
                        )
                        w_ring[i].dma_start(
                            w_f[:], w_drams[i].ap().rearrange("(c p) e -> p c e", p=128)
                        )
                        w_sb = persist.tile(
                            [128, DC, E], BF16, tag=w_names[i], name=w_names[i]
                        )
                        nc.scalar.copy(w_sb[:], w_f[:])
                        w_sbs[i] = w_sb

                xt = [
                    xtp.tile([128, N], FP8, tag=f"xt{dc}", name=f"xt{dc}")
                    for dc in range(DC)
                ]

                def emit_proj(g):
                    sl = slice(g * 512, (g + 1) * 512)
                    for w_sb, b_sb, dstT in zip(w_sbs, b_sbs, projT):
                        pp = pjp.tile([128, 512], F32, tag="pj")
                        for dc in range(DC):
                            nc.tensor.matmul(
                                pp[:],
                                w_sb[:, dc, :],
                                xt[dc][:, sl],
                                start=(dc == 0),
                                stop=(dc == DC - 1),
                            )
                        nc.vector.tensor_scalar_add(dstT[:, sl], pp[:], b_sb[:])

                for g in range(NG):
                    xb = xload.tile([128, 4, D], BF16, tag="xb", bufs=1)
                    for half, ring in ((0, nc.sync), (1, nc.scalar)):
                        xf = xload.tile(
                            [128, 2, D], F32, tag=f"xf{half}", bufs=2,
                            name=f"xf{g}_{half}",
                        )
                        ring.dma_start(xf[:], x_view[g, :, 2 * half : 2 * half + 2])
                        nc.vector.tensor_copy(
                            xb[:, 2 * half : 2 * half + 2, :], xf[:]
                        )
                    sl = slice(g * 512, (g + 1) * 512)
                    for dc in range(DC):
                        ps = trp.tile([128, 4, 128], F32, tag="trps")
                        for j in range(4):
                            nc.tensor.matmul(
                                ps[:, j, :],
                                xb[:, j, dc * 128 : (dc + 1) * 128],
                                eye_bf[:],
                                start=True,
                                stop=True,
                            )
                        nc.vector.tensor_copy(xt[dc][:, sl], ps[:])
                    emit_proj(g)
                    # interleave the early m-chunks' score/exp sweeps under
                    # the X stream: column-chunk ch needs only qT groups
                    # 2ch..2ch+1 and kT groups < SWEEP_MC/4 (all emitted)
                    if g == 1:
                        for mc in range(8):
                            emit_t_exp(sweepTp, mc, [0], "expt0", SWEEP_MC)
                    elif g == 2:
                        for mc in range(8, 12):
                            emit_t_exp(sweepTp, mc, [0], "expt0", SWEEP_MC)
                    elif g == 3:
                        for mc in range(12, SWEEP_MC):
                            emit_t_exp(sweepTp, mc, [0], "expt0", SWEEP_MC)
                        for mc in range(SWEEP_MC):
                            emit_t_exp(sweepTp, mc, [1], "expt0", SWEEP_MC)
                    elif g in (5, 7):
                        ch = (g - 1) // 2
                        for mc in range(SWEEP_MC):
                            emit_t_exp(sweepTp, mc, [ch], "expt0", SWEEP_MC)

                # v [m, e] chunks via transpose matmuls
                for grp in range(8):
                    ps = trp.tile([128, 4, 128], F32, tag="trps", name=f"vtr{grp}")
                    for j in range(4):
                        mc = grp * 4 + j
                        nc.tensor.matmul(
                            ps[:, j, :],
                            vT[:, mc * 128 : (mc + 1) * 128],
                            eye_bf[:],
                            start=True,
                            stop=True,
                        )
                    nc.vector.tensor_copy(v_sb[:, grp * 4 : grp * 4 + 4, :], ps[:])

            # ---------------- block era ----------------
            with (
                tc.tile_pool(name="blockTp", bufs=3, space=PSUM) as blockTp,
                tc.tile_pool(name="avps", bufs=1, space=PSUM) as avp,
                tc.tile_pool(name="outp", bufs=2) as outp,
                tc.tile_pool(name="expt2", bufs=1) as expp2,
            ):
                out_view = out_d.ap().rearrange("(g j p) e -> g p j e", p=128, j=8)

                for mc in range(SWEEP_MC):
                    emit_denom(mc)
                emit_vscale(range(SWEEP_MC))

                def emit_av_part(mc0, sz, nq, first, last):
                    sl = slice(nq * 1024, (nq + 1) * 1024)
                    ap = avp.tile(
                        [128, 1024], F32, tag="av", name=f"av{mc0}_{nq}"
                    )
                    for h2 in range(2):
                        for j in range(sz):
                            mc = mc0 + j
                            nc.tensor.matmul(
                                ap[:, h2 * 512 : (h2 + 1) * 512],
                                vss[mc][:],
                                expts[mc][:, nq * 1024 + h2 * 512 : nq * 1024 + (h2 + 1) * 512],
                                start=(j == 0),
                                stop=(j == sz - 1),
                            )
                    if first:
                        nc.vector.tensor_copy(zT[:, sl], ap[:])
                    else:
                        nc.vector.tensor_tensor(zT[:, sl], zT[:, sl], ap[:], ADD)
                    if last:
                        zsc = outp.tile([128, 1024], BF16, tag="zsc", name=f"zsc{nq}")
                        nc.scalar.activation(zsc[:], zT[:, sl], Sigmoid)
                        ps = blockTp.tile([128, 8, 128], F32, tag="T", name=f"otr{nq}")
                        for j in range(8):
                            nc.tensor.matmul(
                                ps[:, j, :],
                                zsc[:, j * 128 : (j + 1) * 128],
                                eye_bf[:],
                                start=True,
                                stop=True,
                            )
                        ot = outp.tile([128, 8, 128], F32, tag="ot", name=f"ot{nq}")
                        nc.vector.tensor_copy(ot[:], ps[:])
                        nc.sync.dma_start(out_view[nq], ot[:])

                def emit_block(mc0, sz, prevs):
                    """T/exp for this block, with earlier blocks' AV matmuls
                    interleaved per-mc so ACT never starves on the PE FIFO."""
                    for p0, psz in prevs:
                        emit_vscale(range(p0, p0 + psz))
                    parts = [(p0, psz, nq) for p0, psz in prevs for nq in range(4)]
                    done = 0
                    for j in range(sz):
                        mc = mc0 + j
                        emit_t_exp(blockTp, mc, range(CH), "expt", 16, pool=expp2)
                        emit_denom(mc)
                        upto = (j + 1) * len(parts) // sz
                        for p0, psz, nq in parts[done:upto]:
                            emit_av_part(p0, psz, nq, p0 == 0, False)
                        done = upto

                emit_block(14, 6, [])
                emit_block(20, 6, [(0, SWEEP_MC)])
                emit_block(26, 6, [(14, 6), (20, 6)])
                emit_vscale(range(26, 32))
                for nq in range(4):
                    emit_av_part(26, 6, nq, False, True)

    nc.compile()
    return nc


_NC = None


def _get_nc():
    global _NC
    if _NC is None:
        _NC = build()
    return _NC


def _make_in_maps(inputs):
    X = np.ascontiguousarray(np.asarray(inputs["X"], dtype=np.float32))
    Wq = np.asarray(inputs["Wq"], dtype=np.float32)
    Wk = np.asarray(inputs["Wk"], dtype=np.float32)
    Wv = np.asarray(inputs["Wv"], dtype=np.float32)
    bq = np.asarray(inputs["bq"], dtype=np.float32)
    bk = np.asarray(inputs["bk"], dtype=np.float32)
    bv = np.asarray(inputs["bv"], dtype=np.float32)
    in_maps = []
    for h in range(H):
        in_maps.append(
            {
                "x": X,
                "wq": np.ascontiguousarray(Wq[h]),
                "wk": np.ascontiguousarray(Wk[h]),
                "wv": np.ascontiguousarray(Wv[h]),
                "bq": np.ascontiguousarray(bq[h].reshape(E, 1)),
                "bk": np.ascontiguousarray(bk[h].reshape(E, 1)),
                "bv": np.ascontiguousarray(bv[h].reshape(E, 1)),
            }
        )
    return in_maps


def run(inputs, trace=False, tmpdir=None):
    nc = _get_nc()
    res = run_bass_kernel_spmd(
        nc, _make_in_maps(inputs), list(range(H)), trace=trace, tmpdir=tmpdir
    )
    out = np.concatenate([res.results[h]["out"] for h in range(H)], axis=1)
    return out.astype(np.float32), res


def kernel(**inputs) -> np.ndarray:
    out, _ = run(inputs)
    return out


# revision 62
# speedup vs baseline: 1.0577x; 1.0577x over previous
"""Head-parallel multi-head attention kernel for 8 TRN2 NeuronCores.

Problem: X[4096,1024] @ per-head Wq/Wk/Wv[1024,128] (+bias) -> per-head
scores S = q k^T * SCALE, softmax over the QUERY axis (axis n), z = attn @ v,
concat heads, sigmoid.  H=8 heads -> 1 head per core, zero collectives.

Per-core algorithm (head h):
  - Work in transposed space: T = S^T laid out [m(part), n(free)] so the
    softmax reduction is a free-axis row-sum.
  - softmax normalization folded into v:  z^T[e,n] = sum_m exp(T[m,n]) *
    (v[m,e]/denom[m]); denom comes free from the ACT accum_out of the exp.
  - X is transposed on-chip with regular matmuls against identity (fast
    path + engages the HAM clock), XT and expT stored fp8 for SBUF room.
  - The first SWEEP_MC m-chunks are processed column-chunk-outer so their
    exps start while X is still streaming in; the rest run m-chunk-outer
    in blocks whose AV matmuls hide under the next block's exps.
  - zT accumulated in PSUM per block, flushed to SBUF by DVE; sigmoid +
    transpose-back + store fused into the last block's AV sweep.
Host: shard weights per head, replicate X, concat core outputs on axis=1.
"""

import numpy as np
import ml_dtypes

from concourse import bacc, bass, tile, mybir
from concourse.bass_utils import run_bass_kernel_spmd

N, D, E = 4096, 1024, 128
H = 8
SCALE = 0.08838834764831845
BF16 = mybir.dt.bfloat16
F32 = mybir.dt.float32
FP8 = mybir.dt.float8e4

DC = D // 128      # 8 d-chunks of 128 (contraction tiles)
NG = N // 512      # 8 column groups of 512
MC = N // 128      # 32 m-chunks of 128
CH = N // 1024     # 4 exp chunks of 1024 per m-chunk
NQ = N // 512      # 8 AV n-chunks of 512
SWEEP_MC = 12      # m-chunks processed column-outer during the X stream

Exp = mybir.ActivationFunctionType.Exp
Sigmoid = mybir.ActivationFunctionType.Sigmoid
ADD = mybir.AluOpType.add
AX = mybir.AxisListType.X
PSUM = bass.MemorySpace.PSUM


def build():
    nc = bacc.Bacc("TRN2", target_bir_lowering=False, debug=False, num_devices=H)

    x_d = nc.dram_tensor("x", [N, D], F32, kind="ExternalInput")
    wq_d = nc.dram_tensor("wq", [D, E], F32, kind="ExternalInput")
    wk_d = nc.dram_tensor("wk", [D, E], F32, kind="ExternalInput")
    wv_d = nc.dram_tensor("wv", [D, E], F32, kind="ExternalInput")
    bq_d = nc.dram_tensor("bq", [E, 1], F32, kind="ExternalInput")
    bk_d = nc.dram_tensor("bk", [E, 1], F32, kind="ExternalInput")
    bv_d = nc.dram_tensor("bv", [E, 1], F32, kind="ExternalInput")
    out_d = nc.dram_tensor("out", [N, E], F32, kind="ExternalOutput")

    eye_bf_d = nc.inline_tensor(np.eye(128, dtype=ml_dtypes.bfloat16), "eye_bf")

    with tile.TileContext(nc) as tc:
        with (
            tc.tile_pool(name="persist", bufs=1) as persist,
            tc.tile_pool(name="expt", bufs=1) as expp,
            tc.tile_pool(name="vsp", bufs=32) as vsp,
            tc.tile_pool(name="dsp", bufs=1) as dsp,
        ):
            eye_bf = persist.tile([128, 128], BF16, tag="eye_bf")
            nc.gpsimd.dma_start(eye_bf[:], eye_bf_d[:])
            b_sbs = []
            for name, b_d in (("bq", bq_d), ("bk", bk_d), ("bv", bv_d)):
                b_sb = persist.tile([E, 1], F32, tag=name)
                nc.gpsimd.dma_start(b_sb[:], b_d[:])
                b_sbs.append(b_sb)

            qT = persist.tile([E, N], BF16, tag="qT")
            kT = persist.tile([E, N], BF16, tag="kT")
            vT = persist.tile([E, N], BF16, tag="vT")
            v_sb = persist.tile([128, MC, E], BF16, tag="v")
            zT = persist.tile([E, N], F32, tag="zT")
            projT = (qT, kT, vT)

            expts = {}
            recs = {}
            vss = {}

            def emit_t_exp(Tp, mc, chs, etag, ebufs, pool=None):
                """score^T matmuls + exp (fused row-sum) for chunks chs of mc."""
                if mc not in expts:
                    expts[mc] = (pool or expp).tile(
                        [128, N], FP8, tag=etag, name=f"et{mc}", bufs=ebufs
                    )
                    dst = dsp.tile([128, CH], F32, tag="ds", name=f"ds{mc}", bufs=40)
                    recs[mc] = (dst, None)
                et = expts[mc]
                dst = recs[mc][0]
                for ch in chs:
                    tp = Tp.tile([128, 1024], F32, tag="T", name=f"T{mc}_{ch}")
                    for h2 in range(2):
                        nc.tensor.matmul(
                            tp[:, h2 * 512 : (h2 + 1) * 512],
                            kT[:, mc * 128 : (mc + 1) * 128],
                            qT[:, ch * 1024 + h2 * 512 : ch * 1024 + (h2 + 1) * 512],
                            start=True,
                            stop=True,
                        )
                    nc.scalar.activation(
                        et[:, ch * 1024 : (ch + 1) * 1024],
                        tp[:],
                        Exp,
                        scale=SCALE,
                        accum_out=dst[:, ch : ch + 1],
                    )

            def emit_denom(mc):
                dst = recs[mc][0]
                den = dsp.tile([128, 1], F32, tag="den", name=f"den{mc}", bufs=8)
                nc.vector.tensor_reduce(den[:], dst[:], AX, ADD)
                rec = dsp.tile([128, 1], F32, tag="rec", name=f"rec{mc}", bufs=40)
                nc.vector.reciprocal(rec[:], den[:])
                recs[mc] = (dst, rec)

            def emit_vscale(mcs):
                for mc in mcs:
                    vs = vsp.tile([128, E], BF16, tag="vs", name=f"vs{mc}")
                    nc.vector.tensor_scalar_mul(vs[:], v_sb[:, mc, :], recs[mc][1][:])
                    vss[mc] = vs

            # ---------------- phase 1 + sweep era ----------------
            x_view = x_d.ap().rearrange("(g t p) d -> g p t d", p=128, t=4)
            with (
                tc.tile_pool(name="xload", bufs=1) as xload,
                tc.tile_pool(name="xtp", bufs=1) as xtp,
                tc.tile_pool(name="sweepTp", bufs=2, space=PSUM) as sweepTp,
                tc.tile_pool(name="trps", bufs=2, space=PSUM) as trp,
                tc.tile_pool(name="pjps", bufs=2, space=PSUM) as pjp,
            ):
                w_sbs = [None, None, None]
                w_ring = {0: nc.gpsimd, 1: nc.scalar, 2: nc.scalar}
                w_names = ("wq", "wk", "wv")
                w_drams = (wq_d, wk_d, wv_d)
                for i in range(3):
                    w_f = xload.tile(
                        [128, DC, E], F32, tag="wf", name=f"wf{i}", bufs=3
                    )
                    w_ring[i].dma_start(
                        w_f[:], w_drams[i].ap().rearrange("(c p) e -> p c e", p=128)
                    )
                    w_sb = persist.tile(
                        [128, DC, E], BF16, tag=w_names[i], name=w_names[i]
                    )
                    nc.scalar.copy(w_sb[:], w_f[:])
                    w_sbs[i] = w_sb

                xt = [
                    xtp.tile([128, N], FP8, tag=f"xt{dc}", name=f"xt{dc}")
                    for dc in range(DC)
                ]

                def emit_proj(g):
                    sl = slice(g * 512, (g + 1) * 512)
                    for w_sb, b_sb, dstT in zip(w_sbs, b_sbs, projT):
                        pp = pjp.tile([128, 512], F32, tag="pj")
                        for dc in range(DC):
                            nc.tensor.matmul(
                                pp[:],
                                w_sb[:, dc, :],
                                xt[dc][:, sl],
                                start=(dc == 0),
                                stop=(dc == DC - 1),
                            )
                        nc.vector.tensor_scalar_add(dstT[:, sl], pp[:], b_sb[:])

                for g in range(NG):
                    xb = xload.tile([128, 4, D], BF16, tag="xb", bufs=2)
                    for half, ring in ((0, nc.sync), (1, nc.scalar)):
                        xf = xload.tile(
                            [128, 2, D], F32, tag=f"xf{half}", bufs=2,
                            name=f"xf{g}_{half}",
                        )
                        ring.dma_start(xf[:], x_view[g, :, 2 * half : 2 * half + 2])
                        nc.vector.tensor_copy(
                            xb[:, 2 * half : 2 * half + 2, :], xf[:]
                        )
                    sl = slice(g * 512, (g + 1) * 512)
                    for dc in range(DC):
                        ps = trp.tile([128, 4, 128], F32, tag="trps")
                        for j in range(4):
                            nc.tensor.matmul(
                                ps[:, j, :],
                                xb[:, j, dc * 128 : (dc + 1) * 128],
                                eye_bf[:],
                                start=True,
                                stop=True,
                            )
                        nc.vector.tensor_copy(xt[dc][:, sl], ps[:])
                    emit_proj(g)
                    # interleave the early m-chunks' score/exp sweeps under
                    # the X stream: column-chunk ch needs only qT groups
                    # 2ch..2ch+1 and kT groups < SWEEP_MC/4 (all emitted)
                    if g == 1:
                        for mc in range(8):
                            emit_t_exp(sweepTp, mc, [0], "expt0", SWEEP_MC)
                    elif g == 2:
                        for mc in range(8, SWEEP_MC):
                            emit_t_exp(sweepTp, mc, [0], "expt0", SWEEP_MC)
                    elif g in (3, 5, 7):
                        ch = (g - 1) // 2
                        for mc in range(SWEEP_MC):
                            emit_t_exp(sweepTp, mc, [ch], "expt0", SWEEP_MC)

                # v [m, e] chunks via transpose matmuls
                for grp in range(8):
                    ps = trp.tile([128, 4, 128], F32, tag="trps", name=f"vtr{grp}")
                    for j in range(4):
                        mc = grp * 4 + j
                        nc.tensor.matmul(
                            ps[:, j, :],
                            vT[:, mc * 128 : (mc + 1) * 128],
                            eye_bf[:],
                            start=True,
                            stop=True,
                        )
                    nc.vector.tensor_copy(v_sb[:, grp * 4 : grp * 4 + 4, :], ps[:])

            # ---------------- block era ----------------
            with (
                tc.tile_pool(name="blockTp", bufs=3, space=PSUM) as blockTp,
                tc.tile_pool(name="avps", bufs=1, space=PSUM) as avp,
                tc.tile_pool(name="outp", bufs=2) as outp,
                tc.tile_pool(name="expt2", bufs=1) as expp2,
            ):
                out_view = out_d.ap().rearrange("(g j p) e -> g p j e", p=128, j=8)

                for mc in range(SWEEP_MC):
                    emit_denom(mc)
                emit_vscale(range(SWEEP_MC))

                def emit_av_part(mc0, sz, nq, first, last):
                    sl = slice(nq * 1024, (nq + 1) * 1024)
                    ap = avp.tile(
                        [128, 1024], F32, tag="av", name=f"av{mc0}_{nq}"
                    )
                    for h2 in range(2):
                        for j in range(sz):
                            mc = mc0 + j
                            nc.tensor.matmul(
                                ap[:, h2 * 512 : (h2 + 1) * 512],
                                vss[mc][:],
                                expts[mc][:, nq * 1024 + h2 * 512 : nq * 1024 + (h2 + 1) * 512],
                                start=(j == 0),
                                stop=(j == sz - 1),
                            )
                    if first:
                        nc.vector.tensor_copy(zT[:, sl], ap[:])
                    else:
                        nc.vector.tensor_tensor(zT[:, sl], zT[:, sl], ap[:], ADD)
                    if last:
                        zsc = outp.tile([128, 1024], BF16, tag="zsc", name=f"zsc{nq}")
                        nc.scalar.activation(zsc[:], zT[:, sl], Sigmoid)
                        ps = blockTp.tile([128, 8, 128], F32, tag="T", name=f"otr{nq}")
                        for j in range(8):
                            nc.tensor.matmul(
                                ps[:, j, :],
                                zsc[:, j * 128 : (j + 1) * 128],
                                eye_bf[:],
                                start=True,
                                stop=True,
                            )
                        ot = outp.tile([128, 8, 128], F32, tag="ot", name=f"ot{nq}")
                        nc.vector.tensor_copy(ot[:], ps[:])
                        nc.sync.dma_start(out_view[nq], ot[:])

                def emit_block(mc0, sz, prevs):
                    """T/exp for this block, with earlier blocks' AV matmuls
                    interleaved per-mc so ACT never starves on the PE FIFO."""
                    for p0, psz in prevs:
                        emit_vscale(range(p0, p0 + psz))
                    parts = [(p0, psz, nq) for p0, psz in prevs for nq in range(4)]
                    done = 0
                    for j in range(sz):
                        mc = mc0 + j
                        emit_t_exp(blockTp, mc, range(CH), "expt", 16, pool=expp2)
                        emit_denom(mc)
                        upto = (j + 1) * len(parts) // sz
                        for p0, psz, nq in parts[done:upto]:
                            emit_av_part(p0, psz, nq, p0 == 0, False)
                        done = upto

                emit_block(12, 7, [])
                emit_block(19, 7, [(0, SWEEP_MC)])
                emit_block(26, 6, [(12, 7), (19, 7)])
                emit_vscale(range(26, 32))
                for nq in range(4):
                    emit_av_part(26, 6, nq, False, True)

    nc.compile()
    return nc


_NC = None


def _get_nc():
    global _NC
    if _NC is None:
        _NC = build()
    return _NC


def _make_in_maps(inputs):
    X = np.ascontiguousarray(np.asarray(inputs["X"], dtype=np.float32))
    Wq = np.asarray(inputs["Wq"], dtype=np.float32)
    Wk = np.asarray(inputs["Wk"], dtype=np.float32)
    Wv = np.asarray(inputs["Wv"], dtype=np.float32)
    bq = np.asarray(inputs["bq"], dtype=np.float32)
    bk = np.asarray(inputs["bk"], dtype=np.float32)
    bv = np.asarray(inputs["bv"], dtype=np.float32)
    in_maps = []
    for h in range(H):
        in_maps.append(
            {
                "x": X,
                "wq": np.ascontiguousarray(Wq[h]),
                "wk": np.ascontiguousarray(Wk[h]),
                "wv": np.ascontiguousarray(Wv[h]),
                "bq": np.ascontiguousarray(bq[h].reshape(E, 1)),
                "bk": np.ascontiguousarray(bk[h].reshape(E, 1)),
                "bv": np.ascontiguousarray(bv[h].reshape(E, 1)),
            }
        )
    return in_maps


def run(inputs, trace=False, tmpdir=None):
    nc = _get_nc()
    res = run_bass_kernel_spmd(
        nc, _make_in_maps(inputs), list(range(H)), trace=trace, tmpdir=tmpdir
    )
    out = np.concatenate([res.results[h]["out"] for h in range(H)], axis=1)
    return out.astype(np.float32), res


def kernel(**inputs) -> np.ndarray:
    out, _ = run(inputs)
    return out


# revision 63
# speedup vs baseline: 1.0590x; 1.0012x over previous
"""Head-parallel multi-head attention kernel for 8 TRN2 NeuronCores.

Problem: X[4096,1024] @ per-head Wq/Wk/Wv[1024,128] (+bias) -> per-head
scores S = q k^T * SCALE, softmax over the QUERY axis (axis n), z = attn @ v,
concat heads, sigmoid.  H=8 heads -> 1 head per core, zero collectives.

Per-core algorithm (head h):
  - Work in transposed space: T = S^T laid out [m(part), n(free)] so the
    softmax reduction is a free-axis row-sum.
  - softmax normalization folded into v:  z^T[e,n] = sum_m exp(T[m,n]) *
    (v[m,e]/denom[m]); denom comes free from the ACT accum_out of the exp.
  - X is transposed on-chip with regular matmuls against identity (fast
    path + engages the HAM clock), XT and expT stored fp8 for SBUF room.
  - The first SWEEP_MC m-chunks are processed column-chunk-outer so their
    exps start while X is still streaming in; the rest run m-chunk-outer
    in blocks whose AV matmuls hide under the next block's exps.
  - zT accumulated in PSUM per block, flushed to SBUF by DVE; sigmoid +
    transpose-back + store fused into the last block's AV sweep.
Host: shard weights per head, replicate X, concat core outputs on axis=1.
"""

import numpy as np
import ml_dtypes

from concourse import bacc, bass, tile, mybir
from concourse.bass_utils import run_bass_kernel_spmd

N, D, E = 4096, 1024, 128
H = 8
SCALE = 0.08838834764831845
BF16 = mybir.dt.bfloat16
F32 = mybir.dt.float32
FP8 = mybir.dt.float8e4

DC = D // 128      # 8 d-chunks of 128 (contraction tiles)
NG = N // 512      # 8 column groups of 512
MC = N // 128      # 32 m-chunks of 128
CH = N // 1024     # 4 exp chunks of 1024 per m-chunk
NQ = N // 512      # 8 AV n-chunks of 512
SWEEP_MC = 12      # m-chunks processed column-outer during the X stream

Exp = mybir.ActivationFunctionType.Exp
Sigmoid = mybir.ActivationFunctionType.Sigmoid
ADD = mybir.AluOpType.add
AX = mybir.AxisListType.X
PSUM = bass.MemorySpace.PSUM


def build():
    nc = bacc.Bacc("TRN2", target_bir_lowering=False, debug=False, num_devices=H)

    x_d = nc.dram_tensor("x", [N, D], F32, kind="ExternalInput")
    wq_d = nc.dram_tensor("wq", [D, E], F32, kind="ExternalInput")
    wk_d = nc.dram_tensor("wk", [D, E], F32, kind="ExternalInput")
    wv_d = nc.dram_tensor("wv", [D, E], F32, kind="ExternalInput")
    bq_d = nc.dram_tensor("bq", [E, 1], F32, kind="ExternalInput")
    bk_d = nc.dram_tensor("bk", [E, 1], F32, kind="ExternalInput")
    bv_d = nc.dram_tensor("bv", [E, 1], F32, kind="ExternalInput")
    out_d = nc.dram_tensor("out", [N, E], F32, kind="ExternalOutput")

    eye_bf_d = nc.inline_tensor(np.eye(128, dtype=ml_dtypes.bfloat16), "eye_bf")

    with tile.TileContext(nc) as tc:
        with (
            tc.tile_pool(name="persist", bufs=1) as persist,
            tc.tile_pool(name="expt", bufs=1) as expp,
            tc.tile_pool(name="vsp", bufs=32) as vsp,
            tc.tile_pool(name="dsp", bufs=1) as dsp,
        ):
            eye_bf = persist.tile([128, 128], BF16, tag="eye_bf")
            nc.gpsimd.dma_start(eye_bf[:], eye_bf_d[:])
            b_sbs = []
            for name, b_d in (("bq", bq_d), ("bk", bk_d), ("bv", bv_d)):
                b_sb = persist.tile([E, 1], F32, tag=name)
                nc.gpsimd.dma_start(b_sb[:], b_d[:])
                b_sbs.append(b_sb)

            qT = persist.tile([E, N], BF16, tag="qT")
            kT = persist.tile([E, N], BF16, tag="kT")
            vT = persist.tile([E, N], BF16, tag="vT")
            v_sb = persist.tile([128, MC, E], BF16, tag="v")
            zT = persist.tile([E, N], F32, tag="zT")
            projT = (qT, kT, vT)

            expts = {}
            recs = {}
            vss = {}

            def emit_t_exp(Tp, mc, chs, etag, ebufs, pool=None):
                """score^T matmuls + exp (fused row-sum) for chunks chs of mc."""
                if mc not in expts:
                    expts[mc] = (pool or expp).tile(
                        [128, N], FP8, tag=etag, name=f"et{mc}", bufs=ebufs
                    )
                    dst = dsp.tile([128, CH], F32, tag="ds", name=f"ds{mc}", bufs=40)
                    recs[mc] = (dst, None)
                et = expts[mc]
                dst = recs[mc][0]
                for ch in chs:
                    tp = Tp.tile([128, 1024], F32, tag="T", name=f"T{mc}_{ch}")
                    for h2 in range(2):
                        nc.tensor.matmul(
                            tp[:, h2 * 512 : (h2 + 1) * 512],
                            kT[:, mc * 128 : (mc + 1) * 128],
                            qT[:, ch * 1024 + h2 * 512 : ch * 1024 + (h2 + 1) * 512],
                            start=True,
                            stop=True,
                        )
                    nc.scalar.activation(
                        et[:, ch * 1024 : (ch + 1) * 1024],
                        tp[:],
                        Exp,
                        scale=SCALE,
                        accum_out=dst[:, ch : ch + 1],
                    )

            def emit_denom(mc):
                dst = recs[mc][0]
                den = dsp.tile([128, 1], F32, tag="den", name=f"den{mc}", bufs=8)
                nc.vector.tensor_reduce(den[:], dst[:], AX, ADD)
                rec = dsp.tile([128, 1], F32, tag="rec", name=f"rec{mc}", bufs=40)
                nc.vector.reciprocal(rec[:], den[:])
                recs[mc] = (dst, rec)

            def emit_vscale(mcs):
                for mc in mcs:
                    vs = vsp.tile([128, E], BF16, tag="vs", name=f"vs{mc}")
                    nc.vector.tensor_scalar_mul(vs[:], v_sb[:, mc, :], recs[mc][1][:])
                    vss[mc] = vs

            # ---------------- phase 1 + sweep era ----------------
            x_view = x_d.ap().rearrange("(g t p) d -> g p t d", p=128, t=4)
            with (
                tc.tile_pool(name="xload", bufs=1) as xload,
                tc.tile_pool(name="xtp", bufs=1) as xtp,
                tc.tile_pool(name="sweepTp", bufs=2, space=PSUM) as sweepTp,
                tc.tile_pool(name="trps", bufs=2, space=PSUM) as trp,
                tc.tile_pool(name="pjps", bufs=2, space=PSUM) as pjp,
            ):
                w_sbs = [None, None, None]
                w_ring = {0: nc.gpsimd, 1: nc.scalar, 2: nc.scalar}
                w_names = ("wq", "wk", "wv")
                w_drams = (wq_d, wk_d, wv_d)
                for i in range(3):
                    w_f = xload.tile(
                        [128, DC, E], F32, tag="wf", name=f"wf{i}", bufs=3
                    )
                    w_ring[i].dma_start(
                        w_f[:], w_drams[i].ap().rearrange("(c p) e -> p c e", p=128)
                    )
                    w_sb = persist.tile(
                        [128, DC, E], BF16, tag=w_names[i], name=w_names[i]
                    )
                    nc.scalar.copy(w_sb[:], w_f[:])
                    w_sbs[i] = w_sb

                xt = [
                    xtp.tile([128, N], FP8, tag=f"xt{dc}", name=f"xt{dc}")
                    for dc in range(DC)
                ]

                def emit_proj(g):
                    sl = slice(g * 512, (g + 1) * 512)
                    for w_sb, b_sb, dstT in zip(w_sbs, b_sbs, projT):
                        pp = pjp.tile([128, 512], F32, tag="pj")
                        for dc in range(DC):
                            nc.tensor.matmul(
                                pp[:],
                                w_sb[:, dc, :],
                                xt[dc][:, sl],
                                start=(dc == 0),
                                stop=(dc == DC - 1),
                            )
                        nc.vector.tensor_scalar_add(dstT[:, sl], pp[:], b_sb[:])

                for g in range(NG):
                    xb = xload.tile([128, 4, D], BF16, tag="xb", bufs=2)
                    for half, ring in ((0, nc.sync), (1, nc.scalar)):
                        xf = xload.tile(
                            [128, 2, D], F32, tag=f"xf{half}", bufs=2,
                            name=f"xf{g}_{half}",
                        )
                        ring.dma_start(xf[:], x_view[g, :, 2 * half : 2 * half + 2])
                        nc.vector.tensor_copy(
                            xb[:, 2 * half : 2 * half + 2, :], xf[:]
                        )
                    sl = slice(g * 512, (g + 1) * 512)
                    for dc in range(DC):
                        ps = trp.tile([128, 4, 128], F32, tag="trps")
                        for j in range(4):
                            nc.tensor.matmul(
                                ps[:, j, :],
                                xb[:, j, dc * 128 : (dc + 1) * 128],
                                eye_bf[:],
                                start=True,
                                stop=True,
                            )
                        nc.vector.tensor_copy(xt[dc][:, sl], ps[:])
                    emit_proj(g)
                    # interleave the early m-chunks' score/exp sweeps under
                    # the X stream: column-chunk ch needs only qT groups
                    # 2ch..2ch+1 and kT groups < SWEEP_MC/4 (all emitted)
                    if g == 1:
                        for mc in range(8):
                            emit_t_exp(sweepTp, mc, [0], "expt0", SWEEP_MC)
                    elif g == 2:
                        for mc in range(8, SWEEP_MC):
                            emit_t_exp(sweepTp, mc, [0], "expt0", SWEEP_MC)
                    elif g in (3, 5, 7):
                        ch = (g - 1) // 2
                        for mc in range(SWEEP_MC):
                            emit_t_exp(sweepTp, mc, [ch], "expt0", SWEEP_MC)

                # v [m, e] chunks via transpose matmuls
                for grp in range(8):
                    ps = trp.tile([128, 4, 128], F32, tag="trps", name=f"vtr{grp}")
                    for j in range(4):
                        mc = grp * 4 + j
                        nc.tensor.matmul(
                            ps[:, j, :],
                            vT[:, mc * 128 : (mc + 1) * 128],
                            eye_bf[:],
                            start=True,
                            stop=True,
                        )
                    nc.vector.tensor_copy(v_sb[:, grp * 4 : grp * 4 + 4, :], ps[:])

            # ---------------- block era ----------------
            with (
                tc.tile_pool(name="blockTp", bufs=3, space=PSUM) as blockTp,
                tc.tile_pool(name="avps", bufs=1, space=PSUM) as avp,
                tc.tile_pool(name="outp", bufs=2) as outp,
                tc.tile_pool(name="expt2", bufs=1) as expp2,
            ):
                out_view = out_d.ap().rearrange("(g j p) e -> g p j e", p=128, j=8)

                for mc in range(SWEEP_MC):
                    emit_denom(mc)
                emit_vscale(range(SWEEP_MC))

                def emit_av_part(mc0, sz, nq, first, last):
                    sl = slice(nq * 1024, (nq + 1) * 1024)
                    ap = avp.tile(
                        [128, 1024], F32, tag="av", name=f"av{mc0}_{nq}"
                    )
                    for h2 in range(2):
                        for j in range(sz):
                            mc = mc0 + j
                            nc.tensor.matmul(
                                ap[:, h2 * 512 : (h2 + 1) * 512],
                                vss[mc][:],
                                expts[mc][:, nq * 1024 + h2 * 512 : nq * 1024 + (h2 + 1) * 512],
                                start=(j == 0),
                                stop=(j == sz - 1),
                            )
                    if first:
                        nc.vector.tensor_copy(zT[:, sl], ap[:])
                    else:
                        nc.vector.tensor_tensor(zT[:, sl], zT[:, sl], ap[:], ADD)
                    if last:
                        zsc = outp.tile([128, 1024], BF16, tag="zsc", name=f"zsc{nq}")
                        nc.scalar.activation(zsc[:], zT[:, sl], Sigmoid)
                        ps = blockTp.tile([128, 8, 128], F32, tag="T", name=f"otr{nq}")
                        for j in range(8):
                            nc.tensor.matmul(
                                ps[:, j, :],
                                zsc[:, j * 128 : (j + 1) * 128],
                                eye_bf[:],
                                start=True,
                                stop=True,
                            )
                        ot = outp.tile([128, 8, 128], F32, tag="ot", name=f"ot{nq}")
                        nc.vector.tensor_copy(ot[:], ps[:])
                        nc.sync.dma_start(out_view[nq], ot[:])

                def emit_block(mc0, sz, prevs):
                    """T/exp for this block, with earlier blocks' AV matmuls
                    interleaved per-mc so ACT never starves on the PE FIFO."""
                    for p0, psz in prevs:
                        emit_vscale(range(p0, p0 + psz))
                    parts = [(p0, psz, nq) for p0, psz in prevs for nq in range(4)]
                    done = 0
                    for j in range(sz):
                        mc = mc0 + j
                        emit_t_exp(blockTp, mc, range(CH), "expt", 16, pool=expp2)
                        emit_denom(mc)
                        upto = (j + 1) * len(parts) // sz
                        for p0, psz, nq in parts[done:upto]:
                            emit_av_part(p0, psz, nq, p0 == 0, False)
                        done = upto

                emit_block(12, 7, [])
                emit_block(19, 7, [(0, SWEEP_MC)])
                emit_block(26, 3, [(12, 7), (19, 7)])
                emit_block(29, 3, [(26, 3)])
                emit_vscale(range(29, 32))
                for nq in range(4):
                    emit_av_part(29, 3, nq, False, True)

    nc.compile()
    return nc


_NC = None


def _get_nc():
    global _NC
    if _NC is None:
        _NC = build()
    return _NC


def _make_in_maps(inputs):
    X = np.ascontiguousarray(np.asarray(inputs["X"], dtype=np.float32))
    Wq = np.asarray(inputs["Wq"], dtype=np.float32)
    Wk = np.asarray(inputs["Wk"], dtype=np.float32)
    Wv = np.asarray(inputs["Wv"], dtype=np.float32)
    bq = np.asarray(inputs["bq"], dtype=np.float32)
    bk = np.asarray(inputs["bk"], dtype=np.float32)
    bv = np.asarray(inputs["bv"], dtype=np.float32)
    in_maps = []
    for h in range(H):
        in_maps.append(
            {
                "x": X,
                "wq": np.ascontiguousarray(Wq[h]),
                "wk": np.ascontiguousarray(Wk[h]),
                "wv": np.ascontiguousarray(Wv[h]),
                "bq": np.ascontiguousarray(bq[h].reshape(E, 1)),
                "bk": np.ascontiguousarray(bk[h].reshape(E, 1)),
                "bv": np.ascontiguousarray(bv[h].reshape(E, 1)),
            }
        )
    return in_maps


def run(inputs, trace=False, tmpdir=None):
    nc = _get_nc()
    res = run_bass_kernel_spmd(
        nc, _make_in_maps(inputs), list(range(H)), trace=trace, tmpdir=tmpdir
    )
    out = np.concatenate([res.results[h]["out"] for h in range(H)], axis=1)
    return out.astype(np.float32), res


def kernel(**inputs) -> np.ndarray:
    out, _ = run(inputs)
    return out
